# revision 9
# baseline (speedup 1.0000x reference)
"""Trainium2 Bass kernel for nn_Classifier0 (quadrant-sum classifier).

reference:
    agg[n, q]  = quadrant sums of x[n] (512x512, quadrants of 256x256)
    w          = g * v[..., 0] / ||v||            [4, 4]
    y          = agg[:, :, None] * w + b_fgl      [N, 4, 4]
    out        = y.reshape(N, 16) @ W_fc.T + b_fc [N, 10]

Algebraic refactor (exact in real arithmetic):
    out[n, c] = sum_q agg[n, q] * A[q, c] + cc[c]
      A[q, c] = sum_j w[q, j] * W_fc[c, 4q + j]         (4 x 10, host, fp64)
      cc[c]   = b_fgl.ravel() @ W_fc[c] + b_fc[c]       (10, host, fp64)

Device work (data-parallel, 32 samples per core).  The stream is at the
per-NC HBM roofline (~349 GB/s of the ~358 GB/s cap, 716 GB/s per stack
shared by 2 NCs), so the remaining time is all in the edges:

  - bulk: 14 chunks of [128, 4096] (2 samples per chunk, partition p holds
    8 image rows), DVE tensor_reduce sums the left 256 cols per row, ACT
    accum-copy sums the right 256 -> bufL/bufR [128, 14].
  - chunk 0 is DMA'd in two pieces (16 partitions, then 112) so the SDMA
    engines start moving data ~0.5 us earlier (descriptor emission is
    ~5 ns/descriptor, so the 128-descriptor emit otherwise gates the
    stream start).
  - the bulk fc (3 PSUM matmuls + copy + y[0:28] DMA) is issued BEFORE
    the tail chunks so the Tile scheduler's monotonic sem waits fire at
    DVE>=15 (~95 us, hidden under the tail stream) instead of after the
    last tail reduce (the previous kernel lost ~1.5 us to that wait
    coarsening - verified in the NTFF trace).
  - tail: samples 28-30 as [128, 2048]; sample 31 split [128,1536] +
    [128,512] so the final on-critical-path reduce covers 512 floats
    (~0.3 us) instead of 2048 (~1.2 us).  Sample 31's two partial rows
    land in psumB rows 3 and 4 and are merged by a DVE add; the cc row
    is added via a [1,1,1,1,0] ones vector.

Fixed overheads measured in the trace that no kernel in this harness can
avoid: the measured window starts at bass's const-pool memsets (~6.0 us,
before the framework all-engine barrier releases at ~7.2 us), and ends
after walrus's full 256-semaphore teardown sweep (~6.6 us, paced by the
PE sequencer at ~115 ns/clear) plus final drains.
"""

import numpy as np

N, S = 256, 512
H = S // 2
NCORES = 8
SPC = N // NCORES  # samples per core (32)
NCLS = 10

C = 2  # samples per DMA chunk (bulk)
NCH2 = 14  # C=2 chunks per core (samples 0..27)
NT = 4  # single-sample tail samples (28..31)
NTU = 5  # tail units: 28, 29, 30, 31a (1536), 31b (512)
PPS = 128 // C  # partitions per sample in a C=2 chunk (64)
RPP = S // PPS  # image rows per partition (8)
FREE = S * RPP  # floats per partition per C=2 chunk (4096)
FREE1 = S * 4  # floats per partition per C=1 chunk (2048)
T31A = 1536  # sample-31 first piece (3 image rows per partition)
T31B = 512  # sample-31 last piece (1 image row per partition)
HEAD = 16  # partitions in the stream-start head DMA

_PROGRAM_CACHE = {}


def _build_program():
    from contextlib import ExitStack

    import concourse.bacc as bacc
    import concourse.mybir as mybir
    import concourse.tile as tile

    nc = bacc.Bacc("TRN2", target_bir_lowering=False, debug=False)
    dt = mybir.dt.float32

    x_t = nc.dram_tensor("x", [NCH2, 128, FREE], dt, kind="ExternalInput")
    x1_t = nc.dram_tensor("x1", [NT, 128, FREE1], dt, kind="ExternalInput")
    # all folded params packed into one tensor: cols 0:20 walm, 20:40 warm,
    # 40:50 walm1, 50:60 warm1; row 0 cols 60:80 ccbt, 80:90 ccbt1
    cst_t = nc.dram_tensor("cst", [128, 90], dt, kind="ExternalInput")
    y_t = nc.dram_tensor("y", [SPC, NCLS], dt, kind="ExternalOutput")

    with tile.TileContext(nc) as tc, ExitStack() as ctx:
        xpool = ctx.enter_context(tc.tile_pool(name="xp", bufs=8))
        cpool = ctx.enter_context(tc.tile_pool(name="cp", bufs=1))
        ppool = ctx.enter_context(tc.tile_pool(name="pp", bufs=1, space="PSUM"))

        x_ap = x_t.ap()
        x1_ap = x1_t.ap()
        # first 28 y rows viewed as [14 chunks, 20]
        y2 = y_t.ap()[0 : C * NCH2, :].rearrange("(k j) c -> k (j c)", j=C)

        bufL = cpool.tile([128, NCH2], dt)
        bufR = cpool.tile([128, NCH2], dt)
        # tail sums: cols 0..3 = samples 28..31a, col 7 = 31b, cols 4..6
        # zeroed so lhsT=buf[:, 4:8] adds 31b's sums into psum row 3 only
        # (PE can't address psum rows off the 0/32/64 bases directly)
        bufL1 = cpool.tile([128, 8], dt)
        bufR1 = cpool.tile([128, 8], dt)
        nc.vector.memset(bufL1[:, NT : 8 - 1], 0.0)
        nc.vector.memset(bufR1[:, NT : 8 - 1], 0.0)
        # one constant load on the scalar engine's HWDGE ring: the SP ring
        # starts streaming x immediately and GpSimd stays fully idle
        cst = cpool.tile([128, 90], dt)
        nc.scalar.dma_start(cst[:], cst_t.ap())
        walm, warm = cst[:, 0:20], cst[:, 20:40]
        walm1, warm1 = cst[:, 40:50], cst[:, 50:60]
        ccbt, ccbt1 = cst[0:1, 60:80], cst[0:1, 80:90]
        ones1 = cpool.tile([1, NCH2], dt)
        nc.vector.memset(ones1[:], 1.0)

        for k in range(NCH2):
            xt = xpool.tile([128, FREE], dt)
            if k == 0:
                # small head so the SDMA engines start ~0.5 us sooner
                nc.sync.dma_start(xt[0:HEAD, :], x_ap[0][0:HEAD, :])
                nc.sync.dma_start(xt[HEAD:128, :], x_ap[0][HEAD:128, :])
            else:
                nc.sync.dma_start(xt[:], x_ap[k])
            xv = xt[:].rearrange("p (r c) -> p r c", c=S)
            nc.vector.tensor_reduce(
                bufL[:, k : k + 1],
                xv[:, :, 0:H],
                axis=mybir.AxisListType.XY,
                op=mybir.AluOpType.add,
            )
            nc.scalar.activation(
                xv[:, :, H:S],
                xv[:, :, H:S],
                mybir.ActivationFunctionType.Copy,
                accum_out=bufR[:, k : k + 1],
            )

        # bulk fc issued BEFORE the tail so its sem waits fire at DVE>=15
        # (~95 us) and y[0:28] goes out hidden under the tail stream
        psumA = ppool.tile([NCH2, C * NCLS], dt)
        nc.tensor.matmul(psumA[:], lhsT=bufL[:], rhs=walm, start=True, stop=False)
        nc.tensor.matmul(psumA[:], lhsT=bufR[:], rhs=warm, start=False, stop=False)
        nc.tensor.matmul(psumA[:], lhsT=ones1[:], rhs=ccbt, start=False, stop=True)
        outA = cpool.tile([NCH2, C * NCLS], dt)
        nc.vector.tensor_copy(outA[:], psumA[:])

        # single-sample tail chunks; sizes shrink toward the end so the last
        # reduce on the critical path covers only 512 floats per partition
        tails = [
            (x1_ap[0], FREE1),
            (x1_ap[1], FREE1),
            (x1_ap[2], FREE1),
            (x1_ap[3][:, 0:T31A], T31A),
            (x1_ap[3][:, T31A:FREE1], T31B),
        ]
        psumB = ppool.tile([NT, NCLS], dt)
        for k, (src, width) in enumerate(tails):
            col = 7 if k == NT else k
            xt1 = xpool.tile([128, FREE1], dt, tag="x1t")
            nc.sync.dma_start(xt1[:, 0:width], src)
            xv1 = xt1[:, 0:width].rearrange("p (r c) -> p r c", c=S)
            nc.vector.tensor_reduce(
                bufL1[:, col : col + 1],
                xv1[:, :, 0:H],
                axis=mybir.AxisListType.XY,
                op=mybir.AluOpType.add,
            )
            nc.scalar.activation(
                xv1[:, :, H:S],
                xv1[:, :, H:S],
                mybir.ActivationFunctionType.Copy,
                accum_out=bufR1[:, col : col + 1],
            )
            if k == NT - 1:
                # everything except the last 512-float piece is ready: run
                # the bulk of the tail fc now, hidden under the t31b stream
                nc.tensor.matmul(
                    psumB[:], lhsT=bufL1[:, 0:NT], rhs=walm1, start=True, stop=False
                )
                nc.tensor.matmul(
                    psumB[:], lhsT=bufR1[:, 0:NT], rhs=warm1, start=False, stop=False
                )
                nc.tensor.matmul(
                    psumB[:], lhsT=ones1[:, 0:NT], rhs=ccbt1, start=False, stop=False
                )

        # y[0:28] out: issued after the tail DMAs so the SP sequencer never
        # stalls the x stream waiting for outA
        nc.sync.dma_start(y2[:], outA[:])

        # only these two tiny matmuls (t31b's sums into psum row 3 via the
        # zero-padded cols) sit on the critical path after the last byte
        nc.tensor.matmul(
            psumB[:], lhsT=bufL1[:, NT:8], rhs=walm1, start=False, stop=False
        )
        nc.tensor.matmul(
            psumB[:], lhsT=bufR1[:, NT:8], rhs=warm1, start=False, stop=True
        )
        outB = cpool.tile([NT, NCLS], dt)
        nc.vector.tensor_copy(outB[:], psumB[:])
        nc.sync.dma_start(y_t.ap()[C * NCH2 : SPC, :], outB[:])

    nc.compile()
    return nc


def _host_params(v, g, b_fgl, W_fc, b_fc):
    """Fold the tiny params into zero-masked walm/warm [128, C*10], cc [1, C*10]."""
    v64 = v.astype(np.float64)
    w = g.astype(np.float64) * (v64[..., 0] / np.linalg.norm(v64, axis=-1))  # [4,4]
    A = np.einsum("qj,cqj->qc", w, W_fc.astype(np.float64).reshape(NCLS, 4, 4))
    cc = b_fgl.astype(np.float64).reshape(-1) @ W_fc.astype(np.float64).T
    cc = cc + b_fc.astype(np.float64)

    # quadrant ids: 0=TL, 1=BL, 2=BR, 3=TR
    def masks(pps, c):
        p = np.arange(128)
        top = (p % pps) < (pps // 2)
        al_col = np.where(top[:, None], A[0][None, :], A[1][None, :])
        ar_col = np.where(top[:, None], A[3][None, :], A[2][None, :])
        grp = p // pps
        wl = np.zeros((128, c * NCLS))
        wr = np.zeros((128, c * NCLS))
        for j in range(c):
            sel = grp == j
            wl[sel, j * NCLS : (j + 1) * NCLS] = al_col[sel]
            wr[sel, j * NCLS : (j + 1) * NCLS] = ar_col[sel]
        cb = np.tile(cc, c).reshape(1, c * NCLS)
        return (
            np.ascontiguousarray(wl, dtype=np.float32),
            np.ascontiguousarray(wr, dtype=np.float32),
            np.ascontiguousarray(cb, dtype=np.float32),
        )

    return masks(PPS, C), masks(128, 1)


def _run(inputs, trace=False):
    from concourse.bass_utils import run_bass_kernel_spmd

    if "nc" not in _PROGRAM_CACHE:
        _PROGRAM_CACHE["nc"] = _build_program()
    nc = _PROGRAM_CACHE["nc"]

    x = np.ascontiguousarray(np.asarray(inputs["x"], dtype=np.float32))
    (walm, warm, ccbt), (walm1, warm1, ccbt1) = _host_params(
        np.asarray(inputs["v"], np.float32),
        np.asarray(inputs["g"], np.float32),
        np.asarray(inputs["b_fgl"], np.float32),
        np.asarray(inputs["W_fc"], np.float32),
        np.asarray(inputs["b_fc"], np.float32),
    )

    cst = np.zeros((128, 90), np.float32)
    cst[:, 0:20] = walm
    cst[:, 20:40] = warm
    cst[:, 40:50] = walm1
    cst[:, 50:60] = warm1
    cst[0, 60:80] = ccbt[0]
    cst[0, 80:90] = ccbt1[0]
    x_sh = x.reshape(NCORES, SPC * S * S)
    nb = C * NCH2 * S * S  # floats in the C=2 part
    in_maps = [
        {
            "x": x_sh[i, :nb].reshape(NCH2, 128, FREE),
            "x1": x_sh[i, nb:].reshape(NT, 128, FREE1),
            "cst": cst,
        }
        for i in range(NCORES)
    ]
    res = run_bass_kernel_spmd(nc, in_maps, list(range(NCORES)), trace=trace)
    y = np.concatenate([res.results[i]["y"] for i in range(NCORES)], axis=0)
    return y, res.exec_time_ns


def kernel(**inputs) -> np.ndarray:
    y, _ = _run(inputs, trace=False)
    return y
